# revision 8
# baseline (speedup 1.0000x reference)
"""Causal (cumulative) LayerNorm Trainium2 Bass kernel.

Full-input contract: kernel(inputs, gamma, beta) takes the full
(B=8, K=16000, H=256) f32 tensor, shards batch across 8 NeuronCores
(one sample per core), and returns the full (8, 16000, 256) output.

Per-core algorithm (x is (K, H)):
  rowsum[k]   = sum_h x[k, h]
  rowsumsq[k] = sum_h x[k, h]^2
  csum = cumsum(rowsum); cpow = cumsum(rowsumsq)
  mean[k] = csum[k] / (H*(k+1));  msq[k] = cpow[k] / (H*(k+1))
  var[k] = msq[k] - mean[k]^2
  out[k, h] = gamma[h] * (x[k, h] - mean[k]) / sqrt(var[k] + EPS) + beta[h]

Layout: K = 16000 = 125 tiles x 128 rows, SBUF-resident tile-major as
(128 part, 125 tile, 256 h), row k = t*128 + p at [p, t, :]. Per-row
sums come from one bn_stats per tile (even/odd mean/M2, merged on
gpsimd). The 125 tiles are processed as 5 segments of 25 to pipeline
the prefix-scan: per segment, tile-major stats (128, 25) transpose
(exact PE transpose) to chunk-major (25, 128), scan along free axis
(fp32 vector scan), cross-chunk carry via a (2,) transpose pair with a
running inter-segment carry cell, then per-row scale/bias transpose
back tile-major. Output pass per segment: per-tile affine
(scalar/gpsimd), batched gamma multiply (vector/gpsimd), store. This
overlaps segment s's stores with segment s+1's loads so DMA stays
saturated; DMA (~32.8 MB @ ~358 GB/s/core) is the roofline.
"""

import numpy as np

import concourse.bass as bass
import concourse.bacc as bacc
import concourse.tile as tile
from concourse import mybir
from concourse.bass_utils import run_bass_kernel_spmd

EPS = 1e-8
B, K, H = 8, 16000, 256
P = 128                 # SBUF partitions
NT = K // P             # 125 row-tiles per sample
G = 5                   # tiles per DMA group
NG = NT // G            # 25 DMA groups
TS = 25                 # tiles per scan segment
SEG = NT // TS          # 5 segments
GPS = TS // G           # DMA groups per segment
F32 = mybir.dt.float32
ALU = mybir.AluOpType
ACTF = mybir.ActivationFunctionType


def _aff_eng(nc, g):
    # affine engine per group: mostly scalar(ACT), some gpsimd
    return nc.gpsimd if g % 5 == 4 else nc.scalar


def _gam_eng(nc, g):
    # gamma-mult engine per group: vector or gpsimd
    return nc.vector if g % 3 == 0 else nc.gpsimd


def _build(use_beta: bool):
    nc = bacc.Bacc("TRN2", target_bir_lowering=False, debug=False)

    x = nc.declare_dram_parameter("x", [K, H], F32, isOutput=False)
    gamma_b = nc.declare_dram_parameter("gamma_b", [P, H], F32, isOutput=False)
    beta_b = (
        nc.declare_dram_parameter("beta_b", [P, H], F32, isOutput=False)
        if use_beta
        else None
    )
    ident = nc.declare_dram_parameter("ident", [P, P], F32, isOutput=False)
    invc_m = nc.declare_dram_parameter("invc_m", [NT, P], F32, isOutput=False)
    invc_p = nc.declare_dram_parameter("invc_p", [NT, P], F32, isOutput=False)
    y = nc.declare_dram_parameter("y", [K, H], F32, isOutput=True)

    xr = x.rearrange("(t p) h -> p t h", p=P)   # [128, 125, 256]
    yr = y.rearrange("(t p) h -> p t h", p=P)

    with tile.TileContext(nc) as tc:
        with (
            tc.tile_pool(name="singles", bufs=1) as singles,
            tc.tile_pool(name="xpool", bufs=NG) as xpool,
            tc.tile_pool(name="opool", bufs=8) as opool,
            tc.tile_pool(name="segp", bufs=2) as segp,
            tc.tile_pool(name="psum", bufs=1, space="PSUM") as psum,
        ):
            sb_gamma = singles.tile([P, H], F32)
            nc.sync.dma_start(out=sb_gamma[:], in_=gamma_b[:])
            if use_beta:
                sb_beta = singles.tile([P, H], F32)
                nc.sync.dma_start(out=sb_beta[:], in_=beta_b[:])
            sb_ident = singles.tile([P, P], F32)
            nc.sync.dma_start(out=sb_ident[:], in_=ident[:])
            sb_invm = []
            sb_invp = []
            for s in range(SEG):
                tm = singles.tile([TS, P], F32, tag=f"invm{s}")
                nc.sync.dma_start(out=tm[:], in_=invc_m[s * TS:(s + 1) * TS, :])
                sb_invm.append(tm)
                tp_ = singles.tile([TS, P], F32, tag=f"invp{s}")
                nc.sync.dma_start(out=tp_[:], in_=invc_p[s * TS:(s + 1) * TS, :])
                sb_invp.append(tp_)

            sb_eps = singles.tile([P, 1], F32)
            nc.vector.memset(sb_eps[:], EPS)
            carry = singles.tile([2, 1], F32)
            nc.vector.memset(carry[:], 0.0)

            bn = singles.tile([P, NT, 6], F32)   # per-row bn_stats
            inv_t = singles.tile([P, NT], F32)   # rstd, tile-major
            nmi_t = singles.tile([P, NT], F32)   # -mean*rstd, tile-major

            gamma_bc = sb_gamma[:].rearrange("p (o h) -> p o h", o=1).to_broadcast(
                (P, G, H)
            )
            if use_beta:
                beta_bc = sb_beta[:].rearrange("p (o h) -> p o h", o=1).to_broadcast(
                    (P, G, H)
                )

            xtiles = []

            def load_seg(s):
                for gg in range(GPS):
                    g = s * GPS + gg
                    xt = xpool.tile([P, G, H], F32)
                    nc.sync.dma_start(out=xt[:], in_=xr[:, g * G:(g + 1) * G, :])
                    xtiles.append(xt)
                    for j in range(G):
                        t = g * G + j
                        nc.vector.bn_stats(out=bn[:, t, :], in_=xt[:, j, :])

            def scan_seg(s):
                t0 = s * TS
                t1 = t0 + TS
                me = bn[:, t0:t1, 1]
                mo = bn[:, t0:t1, 4]
                m2e = bn[:, t0:t1, 2]
                m2o = bn[:, t0:t1, 5]
                # merge even/odd halves: rowsum/128 and rowsumsq
                se = segp.tile([P, TS], F32)
                nc.vector.tensor_add(out=se[:], in0=me, in1=mo)
                m2 = segp.tile([P, TS], F32)
                nc.vector.tensor_add(out=m2[:], in0=m2e, in1=m2o)
                pe = segp.tile([P, TS], F32)
                nc.vector.tensor_mul(out=pe[:], in0=me, in1=me)
                po = segp.tile([P, TS], F32)
                nc.vector.tensor_mul(out=po[:], in0=mo, in1=mo)
                nc.vector.tensor_add(out=pe[:], in0=pe[:], in1=po[:])
                sp = segp.tile([P, TS], F32)
                nc.vector.scalar_tensor_tensor(
                    out=sp[:], in0=pe[:], scalar=128.0, in1=m2[:],
                    op0=ALU.mult, op1=ALU.add,
                )

                # tile-major -> chunk-major, prefix scans
                ps_s = psum.tile([TS, P], F32)
                nc.tensor.transpose(ps_s[:], se[:], sb_ident[:])
                ps_p = psum.tile([TS, P], F32)
                nc.tensor.transpose(ps_p[:], sp[:], sb_ident[:])
                scan_s = segp.tile([TS, P], F32)
                nc.vector.tensor_tensor_scan(
                    out=scan_s[:], data0=ps_s[:], data1=sb_invm[s][:],
                    initial=0.0, op0=ALU.add, op1=ALU.bypass,
                )
                scan_p = segp.tile([TS, P], F32)
                nc.vector.tensor_tensor_scan(
                    out=scan_p[:], data0=ps_p[:], data1=sb_invm[s][:],
                    initial=0.0, op0=ALU.add, op1=ALU.bypass,
                )

                # cross-chunk exclusive carry (seeded by inter-segment carry)
                tot = segp.tile([TS, 2], F32)
                nc.vector.tensor_copy(out=tot[:, 0:1], in_=scan_s[:, P - 1:P])
                nc.vector.tensor_copy(out=tot[:, 1:2], in_=scan_p[:, P - 1:P])
                pt = psum.tile([2, TS], F32)
                nc.tensor.transpose(pt[:], tot[:], sb_ident[0:TS, 0:TS])
                excl = segp.tile([2, TS], F32)
                nc.vector.tensor_copy(out=excl[:, 0:1], in_=carry[:])
                nc.vector.tensor_tensor_scan(
                    out=excl[:, 1:TS], data0=pt[:, 0:TS - 1],
                    data1=sb_invm[s][0:2, 0:TS - 1],
                    initial=carry[:], op0=ALU.add, op1=ALU.bypass,
                )
                # carry += segment total
                nc.vector.tensor_add(
                    out=carry[:], in0=excl[:, TS - 1:TS], in1=pt[:, TS - 1:TS],
                )
                ps_o = psum.tile([TS, 2], F32)
                nc.tensor.transpose(ps_o[:], excl[:], sb_ident[0:2, 0:2])

                # mean / msq / var / rstd / -mean*rstd  (chunk-major)
                mean_c = segp.tile([TS, P], F32)
                nc.vector.scalar_tensor_tensor(
                    out=mean_c[:], in0=scan_s[:], scalar=ps_o[:, 0:1],
                    in1=sb_invm[s][:], op0=ALU.add, op1=ALU.mult,
                )
                msq_c = segp.tile([TS, P], F32)
                nc.vector.scalar_tensor_tensor(
                    out=msq_c[:], in0=scan_p[:], scalar=ps_o[:, 1:2],
                    in1=sb_invp[s][:], op0=ALU.add, op1=ALU.mult,
                )
                var_c = segp.tile([TS, P], F32)
                nc.vector.tensor_mul(out=var_c[:], in0=mean_c[:], in1=mean_c[:])
                nc.vector.tensor_sub(out=var_c[:], in0=msq_c[:], in1=var_c[:])
                sd_c = segp.tile([TS, P], F32)
                nc.scalar.activation(
                    out=sd_c[:], in_=var_c[:], func=ACTF.Sqrt,
                    bias=sb_eps[0:TS, :],
                )
                inv_c = segp.tile([TS, P], F32)
                nc.vector.reciprocal(out=inv_c[:], in_=sd_c[:])
                nmi_c = segp.tile([TS, P], F32)
                nc.vector.scalar_tensor_tensor(
                    out=nmi_c[:], in0=mean_c[:], scalar=-1.0, in1=inv_c[:],
                    op0=ALU.mult, op1=ALU.mult,
                )

                # back to tile-major
                ps_inv = psum.tile([P, TS], F32)
                nc.tensor.transpose(ps_inv[:], inv_c[:], sb_ident[0:TS, 0:TS])
                ps_nmi = psum.tile([P, TS], F32)
                nc.tensor.transpose(ps_nmi[:], nmi_c[:], sb_ident[0:TS, 0:TS])
                nc.scalar.copy(out=inv_t[:, t0:t1], in_=ps_inv[:])
                nc.scalar.copy(out=nmi_t[:, t0:t1], in_=ps_nmi[:])

            def out_seg(s):
                for gg in range(GPS):
                    g = s * GPS + gg
                    ob = opool.tile([P, G, H], F32)
                    xt = xtiles[g]
                    for j in range(G):
                        t = g * G + j
                        nc.scalar.activation(
                            out=ob[:, j, :], in_=xt[:, j, :],
                            func=ACTF.Identity,
                            bias=nmi_t[:, t:t + 1], scale=inv_t[:, t:t + 1],
                        )
                    nc.gpsimd.tensor_mul(out=ob[:], in0=ob[:], in1=gamma_bc)
                    if use_beta:
                        nc.gpsimd.tensor_add(out=ob[:], in0=ob[:], in1=beta_bc)
                    # stores ride the ACT HWDGE queue so they never block loads
                    nc.scalar.dma_start(out=yr[:, g * G:(g + 1) * G, :], in_=ob[:])

            # software-pipelined emission: phase3 lags one segment
            load_seg(0)
            scan_seg(0)
            for s in range(1, SEG):
                load_seg(s)
                out_seg(s - 1)
                scan_seg(s)
            out_seg(SEG - 1)

    nc.compile()
    return nc


_CACHE = {}


def _get(use_beta: bool):
    if use_beta not in _CACHE:
        _CACHE[use_beta] = _build(use_beta)
    return _CACHE[use_beta]


def _make_consts():
    ident = np.eye(P, dtype=np.float32)
    counts = np.arange(K, dtype=np.float64) + 1.0
    invc_m = (1.0 / (2.0 * counts)).reshape(NT, P).astype(np.float32)
    invc_p = (1.0 / (float(H) * counts)).reshape(NT, P).astype(np.float32)
    return ident, invc_m, invc_p


def _prepare(inputs, gamma, beta):
    inputs = np.ascontiguousarray(inputs, dtype=np.float32)
    gamma = np.asarray(gamma, dtype=np.float32).reshape(1, H)
    beta = np.asarray(beta, dtype=np.float32).reshape(1, H)
    use_beta = bool(np.any(beta))

    gamma_b = np.ascontiguousarray(np.broadcast_to(gamma, (P, H)))
    ident, invc_m, invc_p = _make_consts()

    in_maps = []
    for b in range(B):
        m = {
            "x": np.ascontiguousarray(inputs[b]),
            "gamma_b": gamma_b,
            "ident": ident,
            "invc_m": invc_m,
            "invc_p": invc_p,
        }
        if use_beta:
            m["beta_b"] = np.ascontiguousarray(np.broadcast_to(beta, (P, H)))
        in_maps.append(m)
    return use_beta, in_maps


def kernel(inputs: np.ndarray, gamma: np.ndarray, beta: np.ndarray) -> np.ndarray:
    use_beta, in_maps = _prepare(inputs, gamma, beta)
    nc = _get(use_beta)
    res = run_bass_kernel_spmd(nc, in_maps, list(range(B)))
    out = np.stack([res.results[b]["y"] for b in range(B)], axis=0)
    return out
